# revision 29
# baseline (speedup 1.0000x reference)
"""Criss-cross attention (width=1) Trainium2 Bass kernel.

Math note: for width=1 the criss-cross module collapses to plain unmasked
softmax attention.  The diagonal of energy_H is masked to -inf, but the
"width" logit energy_W[i] equals that same diagonal value q_i.k_i, and it is
re-appended as the (n+1)-th softmax entry.  So per query i the softmax runs
over exactly {q_i.k_j : j=0..n-1}, and

    out = gamma * (V @ softmax_j(Q^T K)) + x
    Q = relu(bn1(w_q x)),  K = relu(bn2(w_k x)),  V = relu(bn3(w_v x))

Sharding: 8 cores = (4 batches) x (2 query halves).  Each core computes
K, V for all 4096 keys of its batch and attention output for its 2048
queries.  Zero cross-core communication.

Per-core structure (matmuls bf16-in / f32-psum):
  S^T[j,i] = K^T Q, computed pair-packed: two K row-groups (K=32
  contraction each) run concurrently via tile_position, filling a
  [128, 1024] PSUM pair tile (two key blocks x 512 queries).
  E = exp(S^T) -> bf16 (no max subtraction needed: logits <= ~30).
  O^T[i, 0:256] accumulates E_blk^T @ [V^T | 1]; the ones column makes
  O^T[i, 256] = Z_i.  Epilogue: per-partition reciprocal, scale (gamma
  pre-folded into V^T), DMA-transpose back to [c, i], residual add.
  The j-loop is software-pipelined: PE runs PV of pair p-1 while ACT
  exponentiates pair p, keeping the tensor engine dense (HAM-warm).
"""

import os
import numpy as np
import ml_dtypes

_B, _C, _N, _CR = 4, 256, 4096, 32
_NCORES = 8
_HALF = _N // 2  # queries per core
_EPS = 1e-5

_BUILD_CACHE: dict = {}


def _build(has_bq: bool, has_bk: bool, has_bv: bool):
    import concourse.mybir as mybir
    import concourse.tile as tile
    from concourse import bacc

    f32 = mybir.dt.float32
    bf16 = mybir.dt.bfloat16
    AF = mybir.ActivationFunctionType
    ALU = mybir.AluOpType

    nc = bacc.Bacc("TRN2", target_bir_lowering=False, debug=False)

    x_d = nc.dram_tensor("x", [_C, _N], f32, kind="ExternalInput")
    xq_d = nc.dram_tensor("xq", [_C, _HALF], f32, kind="ExternalInput")
    wq_d = nc.dram_tensor("wqt4", [_C, 4 * _CR], bf16, kind="ExternalInput")
    wk_d = nc.dram_tensor("wkt4", [_C, 4 * _CR], bf16, kind="ExternalInput")
    wv_d = nc.dram_tensor("wvt", [_C, _C], bf16, kind="ExternalInput")
    g_d = nc.dram_tensor("gvec", [128, 1], f32, kind="ExternalInput")
    bq_d = nc.dram_tensor("bq4", [4 * _CR, 1], f32, kind="ExternalInput") if has_bq else None
    bk_d = nc.dram_tensor("bk4", [4 * _CR, 1], f32, kind="ExternalInput") if has_bk else None
    bv_d = nc.dram_tensor("bv", [1, _C], bf16, kind="ExternalInput") if has_bv else None
    out_d = nc.dram_tensor("out", [_C, _HALF], f32, kind="ExternalOutput")

    NJ = _N // 128        # 32 key blocks
    NP = NJ // 2          # 16 key pairs
    NI5 = _HALF // 512    # 4 query super-blocks
    VTW = _C + 1          # 257: V^T columns + ones column for Z

    with tile.TileContext(nc) as tc:
        with tc.tile_pool(name="persist", bufs=1) as pers, \
             tc.tile_pool(name="work", bufs=2) as work:
            # ---- persistent SBUF tensors ----
            g_sb = pers.tile([128, 1], f32, name="g_sb")
            nc.sync.dma_start(g_sb, g_d.ap())

            ident = pers.tile([128, 128], bf16, name="ident")
            from concourse.masks import make_identity
            make_identity(nc, ident)

            wq_sb = pers.tile([128, 8 * _CR], bf16, name="wq_sb")
            nc.sync.dma_start(wq_sb[:, 0:4 * _CR], wq_d.ap()[0:128, :])
            nc.sync.dma_start(wq_sb[:, 4 * _CR:8 * _CR], wq_d.ap()[128:256, :])
            wk_sb = pers.tile([128, 8 * _CR], bf16, name="wk_sb")
            nc.sync.dma_start(wk_sb[:, 0:4 * _CR], wk_d.ap()[0:128, :])
            nc.sync.dma_start(wk_sb[:, 4 * _CR:8 * _CR], wk_d.ap()[128:256, :])
            wv_sb = pers.tile([128, 2 * _C], bf16, name="wv_sb")
            nc.sync.dma_start(wv_sb[:, 0:_C], wv_d.ap()[0:128, :])
            nc.sync.dma_start(wv_sb[:, _C:2 * _C], wv_d.ap()[128:256, :])

            if has_bq:
                bq_sb = pers.tile([4 * _CR, 1], f32, name="bq_sb")
                nc.sync.dma_start(bq_sb, bq_d.ap())
            if has_bk:
                bk_sb = pers.tile([4 * _CR, 1], f32, name="bk_sb")
                nc.sync.dma_start(bk_sb, bk_d.ap())
            if has_bv:
                bv_sb = pers.tile([1, _C], bf16, name="bv_sb")
                nc.sync.dma_start(bv_sb, bv_d.ap())
                ones_row = pers.tile([1, 128], bf16, name="ones_row")
                nc.any.memset(ones_row, 1.0)

            xbf0 = pers.tile([128, _N], bf16, name="xbf0")
            xbf1 = pers.tile([128, _N], bf16, name="xbf1")
            xq0 = pers.tile([128, _HALF], f32, name="xq0")
            xq1 = pers.tile([128, _HALF], f32, name="xq1")
            xqbf0 = pers.tile([128, _HALF], bf16, name="xqbf0")
            xqbf1 = pers.tile([128, _HALF], bf16, name="xqbf1")
            # quad-packed K: row group t in {0..3}, k_pk[32t+d, g*128+jj]
            # = k[d, (4g+t)*128+jj]
            k_pk = pers.tile([128, (NJ // 4) * 128], bf16, name="k_pk")
            # q replicated in all four row groups
            q_rep = pers.tile([128, _HALF], bf16, name="q_rep")
            vt_sb = pers.tile([128, NJ * VTW], bf16, name="vt_sb")

            # ---- build helpers ----
            def warmup(pps):
                # Keep the tensor engine busy through the initial DMA window
                # so HAM un-throttles before the real matmuls start; results
                # are never read.
                junk = pers.tile([128, 512], bf16, name="junk")
                nc.gpsimd.memset(junk, 0.0)
                warm_ps = pps.tile([128, 512], f32, name="warm_ps", tag="st",
                                   bufs=2)
                for _ in range(48):
                    nc.tensor.matmul(warm_ps, ident, junk, start=True, stop=True)

            def load_xq_and_q(pps):
                # interleave the first x chunks with xq so neither the first
                # QK (needs q) nor the first K/V chunk (needs x) stalls
                for ch in range(2):
                    sl = slice(ch * 512, (ch + 1) * 512)
                    xf0 = work.tile([128, 512], f32, name="xf0", tag="xf", bufs=4)
                    nc.sync.dma_start(xf0, x_d.ap()[0:128, sl])
                    nc.vector.tensor_copy(xbf0[:, sl], xf0)
                    xf1 = work.tile([128, 512], f32, name="xf1", tag="xf", bufs=4)
                    nc.gpsimd.dma_start(xf1, x_d.ap()[128:256, sl])
                    nc.vector.tensor_copy(xbf1[:, sl], xf1)
                for ch in range(_HALF // 1024):
                    sl = slice(ch * 1024, (ch + 1) * 1024)
                    nc.sync.dma_start(xq0[:, sl], xq_d.ap()[0:128, sl])
                    nc.vector.tensor_copy(xqbf0[:, sl], xq0[:, sl])
                    nc.gpsimd.dma_start(xq1[:, sl], xq_d.ap()[128:256, sl])
                    nc.vector.tensor_copy(xqbf1[:, sl], xq1[:, sl])
                for b5 in range(_HALF // 512):
                    sl = slice(b5 * 512, (b5 + 1) * 512)
                    qp = pps.tile([128, 512], f32, name="qp", tag="st", bufs=2)
                    nc.tensor.matmul(qp, wq_sb[:, 0:4 * _CR], xqbf0[:, sl],
                                     start=True, stop=False)
                    nc.tensor.matmul(qp, wq_sb[:, 4 * _CR:8 * _CR], xqbf1[:, sl],
                                     start=False, stop=True)
                    if has_bq:
                        nc.vector.tensor_scalar(q_rep[:, sl], qp, bq_sb, 0.0,
                                                ALU.add, ALU.max)
                    else:
                        nc.vector.tensor_scalar_max(q_rep[:, sl], qp, 0.0)

            def prep_chunk(pps, b5, half):
                """Load/cast x columns [b5*512, (b5+1)*512), compute K quad
                b5 (key blocks 4*b5..4*b5+3) and the same V^T blocks.
                Split into two halves so the PE burst is spread over two
                attention-pair iterations (smoother PE/ACT overlap)."""
                sl = slice(b5 * 512, (b5 + 1) * 512)
                if half == 1:
                    for jb in range(4 * b5 + 2, 4 * b5 + 4):
                        _vt_block(pps, sl, jb)
                    return
                lb = b5 + 2  # chunks 0-1 preloaded alongside xq; stay 2 ahead
                if lb < _N // 512:
                    lsl = slice(lb * 512, (lb + 1) * 512)
                    xf0 = work.tile([128, 512], f32, name="xf0", tag="xf", bufs=4)
                    nc.sync.dma_start(xf0, x_d.ap()[0:128, lsl])
                    nc.vector.tensor_copy(xbf0[:, lsl], xf0)
                    xf1 = work.tile([128, 512], f32, name="xf1", tag="xf", bufs=4)
                    nc.gpsimd.dma_start(xf1, x_d.ap()[128:256, lsl])
                    nc.vector.tensor_copy(xbf1[:, lsl], xf1)

                kp = pps.tile([128, 512], f32, name="kp", tag="st", bufs=2)
                nc.tensor.matmul(kp, wk_sb[:, 0:4 * _CR], xbf0[:, sl],
                                 start=True, stop=False)
                nc.tensor.matmul(kp, wk_sb[:, 4 * _CR:8 * _CR], xbf1[:, sl],
                                 start=False, stop=True)
                for t in range(4):
                    dst = k_pk[32 * t:32 * t + 32, b5 * 128:(b5 + 1) * 128]
                    src = kp[32 * t:32 * t + 32, t * 128:(t + 1) * 128]
                    if has_bk:
                        nc.vector.tensor_scalar(
                            dst, src, bk_sb[32 * t:32 * t + 32, :], 0.0,
                            ALU.add, ALU.max)
                    else:
                        nc.vector.tensor_scalar_max(dst, src, 0.0)
                for jb in range(4 * b5, 4 * b5 + 2):
                    _vt_block(pps, sl, jb)

            def _vt_block(pps, sl, jb):
                    jsl = slice(jb * 128, (jb + 1) * 128)
                    vp = pps.tile([128, _C], f32, name="vp", tag="st", bufs=2)
                    nc.tensor.matmul(vp, xbf0[:, jsl], wv_sb[:, 0:_C], start=True,
                                     stop=not has_bv)
                    nc.tensor.matmul(vp, xbf1[:, jsl], wv_sb[:, _C:2 * _C],
                                     start=False, stop=not has_bv)
                    if has_bv:
                        nc.tensor.matmul(vp, ones_row, bv_sb, start=False, stop=True)
                    vsl = slice(jb * VTW, jb * VTW + _C)
                    nc.vector.tensor_scalar(vt_sb[:, vsl], vp, 0.0, g_sb,
                                            ALU.max, ALU.mult)
                    nc.any.memset(vt_sb[:, jb * VTW + _C:(jb + 1) * VTW], 1.0)

            # ---- attention (software-pipelined over key pairs); prep for
            # x-chunk b5 is interleaved into the first query block so the
            # tensor engine stays dense from the start ----
            with tc.tile_pool(name="att_ps", space="PSUM", bufs=1) as aps:
                pps = aps  # prep PSUM tiles share the "st" tag slots
                warmup(pps)
                load_xq_and_q(pps)

                for i5 in range(NI5):
                    isl = slice(i5 * 512, (i5 + 1) * 512)
                    ots = [
                        aps.tile([128, VTW], f32, name=f"ot{s}", tag=f"ot{s}", bufs=1)
                        for s in range(4)
                    ]
                    e_tiles = [None] * NP
                    st_odd = [None]

                    def qk_exp(p):
                        if p % 2 == 0:
                            # one quad = 4 key blocks through all 4 PE row
                            # groups concurrently, filling two pair tiles
                            g = p // 2
                            st = aps.tile([128, 1024], f32, name="st",
                                          tag="st", bufs=2)
                            stb = aps.tile([128, 1024], f32, name="stb",
                                           tag="st", bufs=2)
                            for t in range(4):
                                dst = st if t < 2 else stb
                                nc.tensor.matmul(
                                    dst[:, (t % 2) * 512:(t % 2 + 1) * 512],
                                    k_pk[32 * t:32 * t + 32, g * 128:(g + 1) * 128],
                                    q_rep[32 * t:32 * t + 32, isl],
                                    start=True, stop=True,
                                    tile_position=(32 * t, 0),
                                )
                            st_odd[0] = stb
                        else:
                            st = st_odd[0]
                        e = work.tile([128, 1024], bf16, name="e_sb", tag="e", bufs=4)
                        nc.scalar.activation(e, st, AF.Exp)
                        e_tiles[p] = e

                    def pv(p):
                        e = e_tiles[p]
                        for s in range(4):
                            for t in range(2):
                                jb = 2 * p + t
                                nc.tensor.matmul(
                                    ots[s],
                                    e[:, t * 512 + s * 128:t * 512 + (s + 1) * 128],
                                    vt_sb[:, jb * VTW:(jb + 1) * VTW],
                                    start=(jb == 0), stop=(jb == NJ - 1),
                                )
                        e_tiles[p] = None

                    for p in range(NP):
                        if i5 == 0:
                            prep_chunk(pps, p // 2, p % 2)
                        qk_exp(p)
                        if p > 0:
                            pv(p - 1)
                    pv(NP - 1)

                    # epilogue: free the ot accumulator banks as fast as
                    # possible (recip+scale first), then transpose/add/store
                    rzs, onrms = [], []
                    for s in range(4):
                        rz = work.tile([128, 1], f32, name="rz", tag=f"rz{s}",
                                       bufs=2)
                        nc.vector.reciprocal(rz, ots[s][:, _C:_C + 1])
                        onrm = work.tile([128, _C], bf16, name="onrm",
                                         tag=f"onrm{s}", bufs=2)
                        nc.vector.tensor_scalar_mul(onrm, ots[s][:, 0:_C], rz)
                        rzs.append(rz)
                        onrms.append(onrm)
                    last = i5 == NI5 - 1
                    for s in range(4):
                        i0 = i5 * 512 + s * 128
                        for chh in range(2):
                            xq_t = xq0 if chh == 0 else xq1
                            res = work.tile([128, 128], f32, name="res", tag="res",
                                            bufs=4)
                            if last:
                                # PE is idle now and the st PSUM slots are
                                # free; PE transpose beats the ~1.2us
                                # serialized DMA transposes for the tail.
                                tp = aps.tile([128, 128], bf16, name="tp",
                                              tag="st", bufs=2)
                                nc.tensor.transpose(
                                    tp, onrms[s][:, chh * 128:(chh + 1) * 128],
                                    ident)
                                nc.vector.tensor_add(res, tp,
                                                     xq_t[:, i0:i0 + 128])
                            else:
                                tT = work.tile([128, 128], bf16, name="tT",
                                               tag="tT", bufs=4)
                                nc.sync.dma_start(
                                    tT, onrms[s][:, chh * 128:(chh + 1) * 128],
                                    transpose=True)
                                nc.vector.tensor_add(res, tT,
                                                     xq_t[:, i0:i0 + 128])
                            dma_eng = nc.sync if last else nc.gpsimd
                            dma_eng.dma_start(
                                out_d.ap()[chh * 128:(chh + 1) * 128, i0:i0 + 128],
                                res)

    nc.compile()
    return nc


def _get_nc(has_bq, has_bk, has_bv):
    key = (has_bq, has_bk, has_bv)
    if key not in _BUILD_CACHE:
        _BUILD_CACHE[key] = _build(*key)
    return _BUILD_CACHE[key]


def kernel(x, w_q, w_k, w_v,
           bn1_scale, bn1_bias, bn1_mean, bn1_var,
           bn2_scale, bn2_bias, bn2_mean, bn2_var,
           bn3_scale, bn3_bias, bn3_mean, bn3_var,
           gamma, _trace=False):
    from concourse.bass_utils import run_bass_kernel_spmd

    x = np.asarray(x, dtype=np.float32)
    gamma_f = float(np.asarray(gamma).reshape(-1)[0])
    bf = ml_dtypes.bfloat16

    def fold(w, s, b, m, v):
        a = np.asarray(s, np.float32) / np.sqrt(np.asarray(v, np.float32) + _EPS)
        return (np.asarray(w, np.float32) * a[:, None],
                np.asarray(b, np.float32) - np.asarray(m, np.float32) * a)

    wqf, bq = fold(w_q, bn1_scale, bn1_bias, bn1_mean, bn1_var)
    wkf, bk = fold(w_k, bn2_scale, bn2_bias, bn2_mean, bn2_var)
    wvf, bv = fold(w_v, bn3_scale, bn3_bias, bn3_mean, bn3_var)
    has_bq = bool(np.any(bq != 0.0))
    has_bk = bool(np.any(bk != 0.0))
    has_bv = bool(np.any(bv != 0.0))

    nc = _get_nc(has_bq, has_bk, has_bv)

    wqt4 = np.tile(np.ascontiguousarray(wqf.T), (1, 4)).astype(bf)  # [c_in, 4cr]
    wkt4 = np.tile(np.ascontiguousarray(wkf.T), (1, 4)).astype(bf)
    wvt = np.ascontiguousarray(wvf.T).astype(bf)                    # [c_in, c_out]
    gvec = np.full((128, 1), gamma_f, dtype=np.float32)

    in_maps = []
    for core in range(_NCORES):
        b, h = divmod(core, 2)
        m = {
            "x": np.ascontiguousarray(x[b]),
            "xq": np.ascontiguousarray(x[b][:, h * _HALF:(h + 1) * _HALF]),
            "wqt4": wqt4, "wkt4": wkt4, "wvt": wvt, "gvec": gvec,
        }
        if has_bq:
            m["bq4"] = np.ascontiguousarray(np.tile(bq, 4).reshape(4 * _CR, 1))
        if has_bk:
            m["bk4"] = np.ascontiguousarray(np.tile(bk, 4).reshape(4 * _CR, 1))
        if has_bv:
            m["bv"] = np.ascontiguousarray(bv.reshape(1, _C)).astype(bf)
        in_maps.append(m)

    res = run_bass_kernel_spmd(nc, in_maps, core_ids=list(range(_NCORES)),
                               trace=_trace)

    out = np.empty((_B, _C, _N), dtype=np.float32)
    for core in range(_NCORES):
        b, h = divmod(core, 2)
        out[b, :, h * _HALF:(h + 1) * _HALF] = res.results[core]["out"]
    if _trace:
        kernel.last_results = res
    return out
